# revision 21
# baseline (speedup 1.0000x reference)
"""Trainium2 Bass kernel for nn_Attention_31147102831130.

Math (per token): qkv = x@Wqkv+b; per-position attention over the HEADS axis:
  q,k,v: [H=16, Dh=64]; A = softmax(q k^T / sqrt(1024)); o = A v (flat 1024)
  y = o@Wo + bo.

Sharding: pure data-parallel over batch: 8 cores x 2 batches (2048 tokens).

Per-core pipeline (token-major, 16 tiles of 128 tokens):
  PE    : QKV projection (bf16), o transposes, output projection (bf16)
  DVE   : bias adds, score products q_h*k_t + d-tree-reduce, A*V + t-tree-reduce
  ACT   : exp
All weights SBUF-resident. Host pre-permutes Wqkv columns to [Q|K|V] head-major
and pre-transposes x so no on-device transposes are needed for phase 1.
"""

import numpy as np
import ml_dtypes

B, S, C = 16, 1024, 1024
E, H, DH = 1024, 16, 64
NCORES = 8
TOK = B * S // NCORES      # 2048 tokens per core
PT = 128                   # tokens per tile
NT = TOK // PT             # 16 tiles
KC = C // 128              # 8 contraction chunks

_CACHE = {}
import os
_GPMODE = int(os.environ.get("KERNEL_GPSIMD", "0"))
_DENSE = os.environ.get("KERNEL_DENSE", "0") == "1"
_GPTILE = int(os.environ.get("KERNEL_GPTILE", "0"))  # every Nth tile attn on gpsimd (0=off)
_EDIRECT = os.environ.get("KERNEL_EDIRECT", "0") == "1"  # unnormalized E in AV; fold 1/Z into o extraction
_SCAN = os.environ.get("KERNEL_SCAN", "0") == "1"  # segmented-sum via tensor_tensor_scan instead of tree adds
_PIPE = os.environ.get("KERNEL_PIPE", "0") == "1"  # software-pipeline: emit phase1(i+1) before attention(i)
_ZBF = os.environ.get("KERNEL_ZBF", "0") == "1"  # bf16 ex/zr so a_bf normalize runs at 2x
_GPTAIL = int(os.environ.get("KERNEL_GPTAIL", "1"))  # last N tree levels on gpsimd
_GPT_SC = int(os.environ.get("KERNEL_GPT_SC", str(_GPTAIL)))
_GPT_AV = int(os.environ.get("KERNEL_GPT_AV", str(_GPTAIL)))
_NH = int(os.environ.get("KERNEL_NH", "1"))  # number of h-groups for products/trees
_BUFS_QKV = int(os.environ.get("KB_QKV", "2"))
_BUFS_ATTN = int(os.environ.get("KB_ATTN", "2"))
_BUFS_PROD = int(os.environ.get("KB_PROD", "2"))
_BUFS_O = int(os.environ.get("KB_O", "2"))
_BUFS_Y = int(os.environ.get("KB_Y", "2"))
_BUFS_PSQKV = int(os.environ.get("KB_PSQKV", "2"))


def _build_nc():
    import os
    import concourse.bass as bass
    import concourse.mybir as mybir
    from concourse.tile import TileContext
    from concourse.masks import make_identity
    from concourse.bass import ts, ds

    bf16 = mybir.dt.bfloat16
    f32 = mybir.dt.float32
    AF = mybir.ActivationFunctionType
    OP = mybir.AluOpType

    nc = bass.Bass()

    xT_d = nc.declare_dram_parameter("xT", [KC, 128, TOK], bf16, isOutput=False)
    wqkv_d = nc.declare_dram_parameter("wqkv", [KC, 128, 3 * E], bf16, isOutput=False)
    bqkv_d = nc.declare_dram_parameter("bqkv", [1, 3 * E], bf16, isOutput=False)
    wo_d = nc.declare_dram_parameter("wo", [KC, 128, E], bf16, isOutput=False)
    bo_d = nc.declare_dram_parameter("bo", [1, E], bf16, isOutput=False)
    y_d = nc.declare_dram_parameter("y", [TOK, E], f32, isOutput=True)

    with TileContext(nc) as tc:
        with (
            tc.tile_pool(name="wpool", bufs=1) as wp,
            tc.tile_pool(name="qkvpool", bufs=_BUFS_QKV) as qp,
            tc.tile_pool(name="attnpool", bufs=_BUFS_ATTN) as ap_,
            tc.tile_pool(name="opool", bufs=_BUFS_O) as op_,
            tc.tile_pool(name="prodpool", bufs=_BUFS_PROD) as pp,
            tc.tile_pool(name="ypool", bufs=_BUFS_Y) as yp,
            tc.tile_pool(name="psqkv", bufs=_BUFS_PSQKV, space="PSUM") as ps_qkv,
            tc.tile_pool(name="pst", bufs=int(os.environ.get("KB_PST", "2")), space="PSUM") as ps_t,
            tc.tile_pool(name="psy", bufs=int(os.environ.get("KB_PSY", "2")), space="PSUM") as ps_y,
        ):
            # ---- persistent weights ----
            xall = wp.tile([128, KC, TOK], bf16)
            _XSPL = int(os.environ.get("KERNEL_XSPL", "8"))
            _XMODE = os.environ.get("KERNEL_XFIRST", "2")  # 0=x last, 1=x first, 2=x0,W,x1..

            def _load_x(sl):
                nc.sync.dma_start(
                    xall[:, :, ds(sl * (TOK // _XSPL), TOK // _XSPL)],
                    xT_d[:, :, ds(sl * (TOK // _XSPL), TOK // _XSPL)].rearrange(
                        "k p t -> p k t"
                    ),
                )

            if _XMODE == "1":
                for sl in range(_XSPL):
                    _load_x(sl)
            elif _XMODE == "2":
                _load_x(0)
            wqkv_sb = wp.tile([128, KC, 3 * E], bf16)
            _WSPL = int(os.environ.get("KERNEL_WSPL", "12"))
            for sl in range(_WSPL):
                w_ = 3 * E // _WSPL
                nc.sync.dma_start(
                    wqkv_sb[:, :, ds(sl * w_, w_)],
                    wqkv_d[:, :, ds(sl * w_, w_)].rearrange("k p e -> p k e"),
                )
            wo_sb = wp.tile([128, KC, E], bf16)
            _OSPL = int(os.environ.get("KERNEL_OSPL", "2"))
            for sl in range(_OSPL):
                w2 = E // _OSPL
                nc.sync.dma_start(
                    wo_sb[:, :, ds(sl * w2, w2)],
                    wo_d[:, :, ds(sl * w2, w2)].rearrange("k p e -> p k e"),
                )
            bq1 = wp.tile([1, 3 * E], bf16)
            nc.sync.dma_start(bq1, bqkv_d[:, :])
            bo1 = wp.tile([1, E], bf16)
            nc.sync.dma_start(bo1, bo_d[:, :])
            ones = wp.tile([1, 128], bf16)
            nc.vector.memset(ones, 1.0)
            ident = wp.tile([128, 128], bf16)
            make_identity(nc, ident)
            mask_t = wp.tile([128, H], bf16)
            nc.vector.memset(mask_t, 1.0)
            nc.vector.memset(mask_t[:, 0:1], 0.0)
            if _SCAN:
                mask_d = wp.tile([128, DH], bf16)   # 0,1,1,...  resets scan at d=0
                nc.vector.memset(mask_d, 1.0)
                nc.vector.memset(mask_d[:, 0:1], 0.0)
            else:
                mask_d = None
            if _XMODE == "0":
                for sl in range(_XSPL):
                    _load_x(sl)
            elif _XMODE == "2":
                for sl in range(1, _XSPL):
                    _load_x(sl)

            def phase1(i):
                # ---- phase 1: qkv = x @ Wqkv + b   (token-major [tok, 3E])
                qkv = qp.tile([128, 3 * E], bf16)
                for j in range(6):  # 6 x 512 output channels
                    ps = ps_qkv.tile([128, 512], f32)
                    for k in range(KC):
                        nc.tensor.matmul(
                            ps,
                            xall[:, k, ts(i, PT)],
                            wqkv_sb[:, k, ds(j * 512, 512)],
                            start=(k == 0),
                            stop=False,
                        )
                    nc.tensor.matmul(
                        ps,
                        ones[0:1, :],
                        bq1[0:1, ds(j * 512, 512)],
                        start=False,
                        stop=True,
                    )
                    nc.scalar.copy(qkv[:, ds(j * 512, 512)], ps)
                return qkv

            def attention(i, qkv):
                gp_tile = _GPTILE > 0 and (i % _GPTILE) == (_GPTILE - 1)
                HH = H // _NH
                k_ap = qkv[:, E : 2 * E].rearrange("p (u t d) -> p u t d", t=H, u=1)
                v_ap = qkv[:, 2 * E : 3 * E].rearrange("p (t d) -> p d t", t=H)

                # vt copy hoisted: only needs v-columns of qkv; emitting it before
                # the scores chain keeps ACT's in-order stream from stalling the
                # A*V products behind exp.
                vt = ap_.tile([128, DH, H], bf16)  # v as [d, t]
                _vt_eng = os.environ.get("KERNEL_VTENG", "scalar")
                if _vt_eng == "gpsimd":
                    nc.gpsimd.tensor_copy(vt, v_ap)
                elif _vt_eng == "vector":
                    nc.vector.tensor_copy(vt, v_ap)
                elif _vt_eng == "late":
                    vt = None
                else:
                    nc.scalar.copy(vt, v_ap)

                # ---- phase 2a: scores product + tree reduce over d (h-halves)
                ex = ap_.tile([128, H, H], bf16 if (_EDIRECT or _ZBF) else f32)
                for g in range(_NH):
                    q_ap = qkv[:, ds(g * HH * DH, HH * DH)].rearrange(
                        "p (h u d) -> p h u d", h=HH, u=1
                    )
                    psc = pp.tile([128, HH, H, DH], bf16, tag="prod")
                    if _DENSE:
                        kd = qkv[:, E : 2 * E].rearrange(
                            "p (u t d) -> p u t d", t=H, u=1
                        )
                        for hh in range(HH):
                            nc.vector.tensor_mul(
                                psc[:, hh : hh + 1, :, :],
                                q_ap[:, hh : hh + 1, :, :].broadcast_to(
                                    [128, 1, H, DH]
                                ),
                                kd.broadcast_to([128, 1, H, DH]),
                            )
                    else:
                        eng_p = nc.gpsimd if gp_tile else nc.vector
                        eng_p.tensor_mul(
                            psc,
                            q_ap.broadcast_to([128, HH, H, DH]),
                            k_ap.broadcast_to([128, HH, H, DH]),
                        )
                    if _SCAN:
                        nc.vector.tensor_tensor_scan(
                            psc.rearrange("p h t d -> p (h t) d"),
                            mask_d.rearrange("p (u d) -> p u d", u=1).broadcast_to(
                                [128, HH * H, DH]
                            ),
                            psc.rearrange("p h t d -> p (h t) d"),
                            0.0,
                            OP.mult,
                            OP.add,
                        )
                        s_src = psc[:, :, :, DH - 1]
                    else:
                        eng_s = nc.gpsimd if (gp_tile or (g == 1 and _GPMODE in (1, 2))) else nc.vector
                        w = DH
                        while w > 1:
                            w //= 2
                            e_ = nc.gpsimd if (w < (1 << _GPT_SC)) else eng_s
                            e_.tensor_add(
                                psc[:, :, :, 0:w],
                                psc[:, :, :, 0:w],
                                psc[:, :, :, w : 2 * w],
                            )
                        s_src = psc[:, :, :, 0]
                    # softmax exp (no max-sub; |scores/32| is small)
                    nc.scalar.activation(
                        ex[:, ds(g * HH, HH), :],
                        s_src,
                        AF.Exp,
                        scale=float(E) ** -0.5,
                    )
                zr = ap_.tile([128, H], f32)
                nc.vector.reduce_sum(zr, ex, axis=mybir.AxisListType.X)
                if _ZBF and not _EDIRECT:
                    zrb = ap_.tile([128, H], bf16)
                    with nc.allow_low_precision(reason="softmax denom fits bf16"):
                        nc.vector.reciprocal(zrb, zr)
                    a_bf = ap_.tile([128, H, H], bf16)
                    nc.vector.tensor_mul(
                        a_bf,
                        ex,
                        zrb.rearrange("p (h u) -> p h u", u=1).broadcast_to([128, H, H]),
                    )
                elif not _EDIRECT:
                    nc.vector.reciprocal(zr, zr)
                    a_bf = ap_.tile([128, H, H], bf16)
                    nc.vector.tensor_mul(
                        a_bf,
                        ex,
                        zr.rearrange("p (h u) -> p h u", u=1).broadcast_to([128, H, H]),
                    )
                else:
                    nc.vector.reciprocal(zr, zr)
                    a_bf = ex

                # ---- phase 2b: o = A @ v per token: product + tree over t
                o_c = op_.tile([128, E], bf16)  # [tok, (h d)]
                vt_b = vt.rearrange("p (u d) t -> p u d t", u=1)
                for g in range(_NH):
                    pav = pp.tile([128, HH, DH, H], bf16, tag="prod")
                    _a_src = mask_t if os.environ.get("KERNEL_FAKEA", "0") == "1" else a_bf
                    _a_ap = (_a_src.rearrange("p (a b t) -> p a b t", a=1, b=1)
                             .broadcast_to([128, HH, DH, H])) if _a_src is mask_t else (
                        a_bf[:, ds(g * HH, HH), :]
                        .rearrange("p h (u t) -> p h u t", u=1)
                        .broadcast_to([128, HH, DH, H]))
                    (nc.gpsimd if gp_tile else nc.vector).tensor_mul(
                        pav,
                        _a_ap,
                        vt_b.broadcast_to([128, HH, DH, H]),
                    )
                    if _SCAN:
                        nc.vector.tensor_tensor_scan(
                            pav.rearrange("p h d t -> p (h d) t"),
                            mask_t.rearrange("p (u t) -> p u t", u=1).broadcast_to(
                                [128, HH * DH, H]
                            ),
                            pav.rearrange("p h d t -> p (h d) t"),
                            0.0,
                            OP.mult,
                            OP.add,
                        )
                    else:
                        eng_a = nc.gpsimd if (gp_tile or (g == 1 and _GPMODE in (1, 3))) else nc.vector
                        w = H
                        while w > 1:
                            w //= 2
                            e_ = nc.gpsimd if (w < (1 << _GPT_AV)) else eng_a
                            e_.tensor_add(
                                pav[:, :, :, 0:w],
                                pav[:, :, :, 0:w],
                                pav[:, :, :, w : 2 * w],
                            )
                    if _EDIRECT:
                        for hh in range(HH):
                            nc.scalar.activation(
                                o_c[:, ds((g * HH + hh) * DH, DH)],
                                pav[:, hh, :, 0],
                                AF.Copy,
                                scale=zr[:, g * HH + hh : g * HH + hh + 1],
                            )
                    else:
                        _oc_eng = {"gpsimd": nc.gpsimd, "vector": nc.vector}.get(
                            os.environ.get("KERNEL_OCENG", "scalar"), nc.scalar
                        )
                        (_oc_eng.tensor_copy if _oc_eng is not nc.scalar else nc.scalar.copy)(
                            o_c[:, ds(g * HH * DH, HH * DH)].rearrange(
                                "p (h d) -> p h d", h=HH
                            ),
                            pav[:, :, :, H - 1] if _SCAN else pav[:, :, :, 0],
                        )

                # ---- transpose o to channel-major for o_proj
                pst = ps_t.tile([128, KC, 128], bf16)
                oT = op_.tile([128, KC, 128], bf16)
                ysb = yp.tile([128, E], f32)
                if os.environ.get("KERNEL_OPIPE", "0") == "1":
                    # interleave per contraction-half: transpose half -> copy ->
                    # its partial MMs, so the o_proj starts before the full o^T
                    # is materialized (shrinks the last tile's serial tail).
                    psy0 = ps_y.tile([128, 512], f32)
                    psy1 = ps_y.tile([128, 512], f32)
                    for hf in range(2):
                        for m in range(4 * hf, 4 * hf + 4):
                            nc.tensor.transpose(pst[:, m, :], o_c[:, ts(m, 128)], ident)
                        nc.scalar.copy(
                            oT[:, 4 * hf : 4 * hf + 4, :], pst[:, 4 * hf : 4 * hf + 4, :]
                        )
                        for j, psy in ((0, psy0), (1, psy1)):
                            for m in range(4 * hf, 4 * hf + 4):
                                nc.tensor.matmul(
                                    psy,
                                    oT[:, m, :],
                                    wo_sb[:, m, ds(j * 512, 512)],
                                    start=(m == 0),
                                    stop=False,
                                )
                    for j, psy in ((0, psy0), (1, psy1)):
                        nc.tensor.matmul(
                            psy,
                            ones[0:1, :],
                            bo1[0:1, ds(j * 512, 512)],
                            start=False,
                            stop=True,
                        )
                        nc.scalar.copy(ysb[:, ds(j * 512, 512)], psy)
                else:
                    for m in range(KC):
                        nc.tensor.transpose(pst[:, m, :], o_c[:, ts(m, 128)], ident)
                    nc.scalar.copy(oT[:, 0:4, :], pst[:, 0:4, :])
                    nc.scalar.copy(oT[:, 4:8, :], pst[:, 4:8, :])

                    # ---- phase 3: y = o @ Wo + bo
                    for j in range(2):
                        psy = ps_y.tile([128, 512], f32)
                        for m in range(KC):
                            nc.tensor.matmul(
                                psy,
                                oT[:, m, :],
                                wo_sb[:, m, ds(j * 512, 512)],
                                start=(m == 0),
                                stop=False,
                            )
                        nc.tensor.matmul(
                            psy,
                            ones[0:1, :],
                            bo1[0:1, ds(j * 512, 512)],
                            start=False,
                            stop=True,
                        )
                        nc.scalar.copy(ysb[:, ds(j * 512, 512)], psy)
                if os.environ.get("KERNEL_YSPLIT", "1") == "1":
                    nc.sync.dma_start(y_d[ts(i, PT), 0:512], ysb[:, 0:512])
                    nc.sync.dma_start(y_d[ts(i, PT), 512:1024], ysb[:, 512:1024])
                else:
                    nc.sync.dma_start(y_d[ts(i, PT), :], ysb)

            if _PIPE:
                qkv_next = phase1(0)
                for i in range(NT):
                    qkv_cur = qkv_next
                    if i + 1 < NT:
                        qkv_next = phase1(i + 1)
                    attention(i, qkv_cur)
            else:
                for i in range(NT):
                    attention(i, phase1(i))

    _legalize_waits(nc, mybir)
    return nc


def _legalize_waits(nc, mybir):
    """This walrus build allows only ONE sync wait per engine instruction.
    Split extra waits into standalone same-engine EventSemaphore insts."""
    for f in nc.m.functions:
        for b in f.blocks:
            newl = []
            for inst in b.instructions:
                si = getattr(inst, "sync_info", None)
                ow = list(si.on_wait) if si and si.on_wait else []
                if len(ow) > 1:
                    for w in ow[:-1]:
                        newl.append(
                            mybir.InstEventSemaphore(
                                name=f"WS-{nc.next_id()}",
                                engine=inst.engine,
                                sync_info=mybir.SyncInfo(on_wait=[w], on_update=[]),
                            )
                        )
                    si.on_wait = [ow[-1]]
                newl.append(inst)
            b.instructions = newl


def _prep_weights(w_qkv, b_qkv, w_o, b_o):
    # permute fused-qkv columns: orig e = h*192 + part*64 + d
    #                           new  e = part*1024 + h*64 + d
    part, h, d = np.meshgrid(
        np.arange(3), np.arange(H), np.arange(DH), indexing="ij"
    )
    perm = (h * 192 + part * 64 + d).reshape(-1)
    wq = np.ascontiguousarray(w_qkv[:, perm]).astype(ml_dtypes.bfloat16)
    bq = np.ascontiguousarray(b_qkv[perm]).astype(ml_dtypes.bfloat16)[None, :]
    wo = np.ascontiguousarray(w_o).astype(ml_dtypes.bfloat16)
    return (
        wq.reshape(KC, 128, 3 * E),
        bq,
        wo.reshape(KC, 128, E),
        np.asarray(b_o).astype(ml_dtypes.bfloat16)[None, :],
    )


def kernel(x, w_qkv, b_qkv, w_o, b_o):
    from concourse.bass_utils import run_bass_kernel_spmd

    if "nc" not in _CACHE:
        _CACHE["nc"] = _build_nc()
    nc = _CACHE["nc"]

    wq, bq, wo, bo = _prep_weights(
        np.asarray(w_qkv, np.float32),
        np.asarray(b_qkv, np.float32),
        np.asarray(w_o, np.float32),
        np.asarray(b_o, np.float32),
    )
    x = np.asarray(x, np.float32)
    in_maps = []
    for c in range(NCORES):
        xc = x[2 * c : 2 * c + 2].reshape(TOK, C)
        xT = np.ascontiguousarray(xc.T).astype(ml_dtypes.bfloat16)
        in_maps.append(
            {
                "xT": xT.reshape(KC, 128, TOK),
                "wqkv": wq,
                "bqkv": bq,
                "wo": wo,
                "bo": bo,
            }
        )

    res = run_bass_kernel_spmd(nc, in_maps, core_ids=list(range(NCORES)))
    out = np.empty((B, S, E), np.float32)
    for c in range(NCORES):
        out[2 * c : 2 * c + 2] = res.results[c]["y"].reshape(2, S, E)
    return out



# revision 22
# speedup vs baseline: 1.0001x; 1.0001x over previous
"""Trainium2 Bass kernel for nn_Attention_31147102831130.

Math (per token): qkv = x@Wqkv+b; per-position attention over the HEADS axis:
  q,k,v: [H=16, Dh=64]; A = softmax(q k^T / sqrt(1024)); o = A v (flat 1024)
  y = o@Wo + bo.

Sharding: pure data-parallel over batch: 8 cores x 2 batches (2048 tokens).

Per-core pipeline (token-major, 16 tiles of 128 tokens):
  PE    : QKV projection (bf16), o transposes, output projection (bf16)
  DVE   : bias adds, score products q_h*k_t + d-tree-reduce, A*V + t-tree-reduce
  ACT   : exp
All weights SBUF-resident. Host pre-permutes Wqkv columns to [Q|K|V] head-major
and pre-transposes x so no on-device transposes are needed for phase 1.
"""

import numpy as np
import ml_dtypes

B, S, C = 16, 1024, 1024
E, H, DH = 1024, 16, 64
NCORES = 8
TOK = B * S // NCORES      # 2048 tokens per core
PT = 128                   # tokens per tile
NT = TOK // PT             # 16 tiles
KC = C // 128              # 8 contraction chunks

_CACHE = {}
import os
_GPMODE = int(os.environ.get("KERNEL_GPSIMD", "0"))
_DENSE = os.environ.get("KERNEL_DENSE", "0") == "1"
_GPTILE = int(os.environ.get("KERNEL_GPTILE", "0"))  # every Nth tile attn on gpsimd (0=off)
_EDIRECT = os.environ.get("KERNEL_EDIRECT", "0") == "1"  # unnormalized E in AV; fold 1/Z into o extraction
_SCAN = os.environ.get("KERNEL_SCAN", "0") == "1"  # segmented-sum via tensor_tensor_scan instead of tree adds
_PIPE = os.environ.get("KERNEL_PIPE", "0") == "1"  # software-pipeline: emit phase1(i+1) before attention(i)
_ZBF = os.environ.get("KERNEL_ZBF", "0") == "1"  # bf16 ex/zr so a_bf normalize runs at 2x
_GPTAIL = int(os.environ.get("KERNEL_GPTAIL", "1"))  # last N tree levels on gpsimd
_GPT_SC = int(os.environ.get("KERNEL_GPT_SC", str(_GPTAIL)))
_GPT_AV = int(os.environ.get("KERNEL_GPT_AV", str(_GPTAIL)))
_NH = int(os.environ.get("KERNEL_NH", "1"))  # number of h-groups for products/trees
_BUFS_QKV = int(os.environ.get("KB_QKV", "2"))
_BUFS_ATTN = int(os.environ.get("KB_ATTN", "2"))
_BUFS_PROD = int(os.environ.get("KB_PROD", "2"))
_BUFS_O = int(os.environ.get("KB_O", "2"))
_BUFS_Y = int(os.environ.get("KB_Y", "2"))
_BUFS_PSQKV = int(os.environ.get("KB_PSQKV", "2"))


def _build_nc():
    import os
    import concourse.bass as bass
    import concourse.mybir as mybir
    from concourse.tile import TileContext
    from concourse.masks import make_identity
    from concourse.bass import ts, ds

    bf16 = mybir.dt.bfloat16
    f32 = mybir.dt.float32
    AF = mybir.ActivationFunctionType
    OP = mybir.AluOpType

    nc = bass.Bass()

    xT_d = nc.declare_dram_parameter("xT", [KC, 128, TOK], bf16, isOutput=False)
    wqkv_d = nc.declare_dram_parameter("wqkv", [KC, 128, 3 * E], bf16, isOutput=False)
    bqkv_d = nc.declare_dram_parameter("bqkv", [1, 3 * E], bf16, isOutput=False)
    wo_d = nc.declare_dram_parameter("wo", [KC, 128, E], bf16, isOutput=False)
    bo_d = nc.declare_dram_parameter("bo", [1, E], bf16, isOutput=False)
    y_d = nc.declare_dram_parameter("y", [TOK, E], f32, isOutput=True)

    with TileContext(nc) as tc:
        with (
            tc.tile_pool(name="wpool", bufs=1) as wp,
            tc.tile_pool(name="qkvpool", bufs=_BUFS_QKV) as qp,
            tc.tile_pool(name="attnpool", bufs=_BUFS_ATTN) as ap_,
            tc.tile_pool(name="opool", bufs=_BUFS_O) as op_,
            tc.tile_pool(name="prodpool", bufs=_BUFS_PROD) as pp,
            tc.tile_pool(name="ypool", bufs=_BUFS_Y) as yp,
            tc.tile_pool(name="psqkv", bufs=_BUFS_PSQKV, space="PSUM") as ps_qkv,
            tc.tile_pool(name="pst", bufs=int(os.environ.get("KB_PST", "2")), space="PSUM") as ps_t,
            tc.tile_pool(name="psy", bufs=int(os.environ.get("KB_PSY", "2")), space="PSUM") as ps_y,
        ):
            # ---- persistent weights ----
            xall = wp.tile([128, KC, TOK], bf16)
            _XSPL = int(os.environ.get("KERNEL_XSPL", "8"))
            _XMODE = os.environ.get("KERNEL_XFIRST", "2")  # 0=x last, 1=x first, 2=x0,W,x1..

            def _load_x(sl):
                nc.sync.dma_start(
                    xall[:, :, ds(sl * (TOK // _XSPL), TOK // _XSPL)],
                    xT_d[:, :, ds(sl * (TOK // _XSPL), TOK // _XSPL)].rearrange(
                        "k p t -> p k t"
                    ),
                )

            if _XMODE == "1":
                for sl in range(_XSPL):
                    _load_x(sl)
            elif _XMODE == "2":
                _load_x(0)
            wqkv_sb = wp.tile([128, KC, 3 * E], bf16)
            _WSPL = int(os.environ.get("KERNEL_WSPL", "12"))
            for sl in range(_WSPL):
                w_ = 3 * E // _WSPL
                nc.sync.dma_start(
                    wqkv_sb[:, :, ds(sl * w_, w_)],
                    wqkv_d[:, :, ds(sl * w_, w_)].rearrange("k p e -> p k e"),
                )
            wo_sb = wp.tile([128, KC, E], bf16)
            _OSPL = int(os.environ.get("KERNEL_OSPL", "1"))
            for sl in range(_OSPL):
                w2 = E // _OSPL
                nc.sync.dma_start(
                    wo_sb[:, :, ds(sl * w2, w2)],
                    wo_d[:, :, ds(sl * w2, w2)].rearrange("k p e -> p k e"),
                )
            bq1 = wp.tile([1, 3 * E], bf16)
            nc.sync.dma_start(bq1, bqkv_d[:, :])
            bo1 = wp.tile([1, E], bf16)
            nc.sync.dma_start(bo1, bo_d[:, :])
            ones = wp.tile([1, 128], bf16)
            nc.vector.memset(ones, 1.0)
            ident = wp.tile([128, 128], bf16)
            make_identity(nc, ident)
            mask_t = wp.tile([128, H], bf16)
            nc.vector.memset(mask_t, 1.0)
            nc.vector.memset(mask_t[:, 0:1], 0.0)
            if _SCAN:
                mask_d = wp.tile([128, DH], bf16)   # 0,1,1,...  resets scan at d=0
                nc.vector.memset(mask_d, 1.0)
                nc.vector.memset(mask_d[:, 0:1], 0.0)
            else:
                mask_d = None
            if _XMODE == "0":
                for sl in range(_XSPL):
                    _load_x(sl)
            elif _XMODE == "2":
                for sl in range(1, _XSPL):
                    _load_x(sl)

            def phase1(i):
                # ---- phase 1: qkv = x @ Wqkv + b   (token-major [tok, 3E])
                qkv = qp.tile([128, 3 * E], bf16)
                for j in range(6):  # 6 x 512 output channels
                    ps = ps_qkv.tile([128, 512], f32)
                    for k in range(KC):
                        nc.tensor.matmul(
                            ps,
                            xall[:, k, ts(i, PT)],
                            wqkv_sb[:, k, ds(j * 512, 512)],
                            start=(k == 0),
                            stop=False,
                        )
                    nc.tensor.matmul(
                        ps,
                        ones[0:1, :],
                        bq1[0:1, ds(j * 512, 512)],
                        start=False,
                        stop=True,
                    )
                    nc.scalar.copy(qkv[:, ds(j * 512, 512)], ps)
                return qkv

            def attention(i, qkv):
                gp_tile = _GPTILE > 0 and (i % _GPTILE) == (_GPTILE - 1)
                HH = H // _NH
                k_ap = qkv[:, E : 2 * E].rearrange("p (u t d) -> p u t d", t=H, u=1)
                v_ap = qkv[:, 2 * E : 3 * E].rearrange("p (t d) -> p d t", t=H)

                # vt copy hoisted: only needs v-columns of qkv; emitting it before
                # the scores chain keeps ACT's in-order stream from stalling the
                # A*V products behind exp.
                vt = ap_.tile([128, DH, H], bf16)  # v as [d, t]
                _vt_eng = os.environ.get("KERNEL_VTENG", "scalar")
                if _vt_eng == "gpsimd":
                    nc.gpsimd.tensor_copy(vt, v_ap)
                elif _vt_eng == "vector":
                    nc.vector.tensor_copy(vt, v_ap)
                elif _vt_eng == "late":
                    vt = None
                else:
                    nc.scalar.copy(vt, v_ap)

                # ---- phase 2a: scores product + tree reduce over d (h-halves)
                ex = ap_.tile([128, H, H], bf16 if (_EDIRECT or _ZBF) else f32)
                for g in range(_NH):
                    q_ap = qkv[:, ds(g * HH * DH, HH * DH)].rearrange(
                        "p (h u d) -> p h u d", h=HH, u=1
                    )
                    psc = pp.tile([128, HH, H, DH], bf16, tag="prod")
                    if _DENSE:
                        kd = qkv[:, E : 2 * E].rearrange(
                            "p (u t d) -> p u t d", t=H, u=1
                        )
                        for hh in range(HH):
                            nc.vector.tensor_mul(
                                psc[:, hh : hh + 1, :, :],
                                q_ap[:, hh : hh + 1, :, :].broadcast_to(
                                    [128, 1, H, DH]
                                ),
                                kd.broadcast_to([128, 1, H, DH]),
                            )
                    else:
                        eng_p = nc.gpsimd if gp_tile else nc.vector
                        eng_p.tensor_mul(
                            psc,
                            q_ap.broadcast_to([128, HH, H, DH]),
                            k_ap.broadcast_to([128, HH, H, DH]),
                        )
                    if _SCAN:
                        nc.vector.tensor_tensor_scan(
                            psc.rearrange("p h t d -> p (h t) d"),
                            mask_d.rearrange("p (u d) -> p u d", u=1).broadcast_to(
                                [128, HH * H, DH]
                            ),
                            psc.rearrange("p h t d -> p (h t) d"),
                            0.0,
                            OP.mult,
                            OP.add,
                        )
                        s_src = psc[:, :, :, DH - 1]
                    else:
                        eng_s = nc.gpsimd if (gp_tile or (g == 1 and _GPMODE in (1, 2))) else nc.vector
                        w = DH
                        while w > 1:
                            w //= 2
                            e_ = nc.gpsimd if (w < (1 << _GPT_SC)) else eng_s
                            e_.tensor_add(
                                psc[:, :, :, 0:w],
                                psc[:, :, :, 0:w],
                                psc[:, :, :, w : 2 * w],
                            )
                        s_src = psc[:, :, :, 0]
                    # softmax exp (no max-sub; |scores/32| is small)
                    nc.scalar.activation(
                        ex[:, ds(g * HH, HH), :],
                        s_src,
                        AF.Exp,
                        scale=float(E) ** -0.5,
                    )
                zr = ap_.tile([128, H], f32)
                nc.vector.reduce_sum(zr, ex, axis=mybir.AxisListType.X)
                if _ZBF and not _EDIRECT:
                    zrb = ap_.tile([128, H], bf16)
                    with nc.allow_low_precision(reason="softmax denom fits bf16"):
                        nc.vector.reciprocal(zrb, zr)
                    a_bf = ap_.tile([128, H, H], bf16)
                    nc.vector.tensor_mul(
                        a_bf,
                        ex,
                        zrb.rearrange("p (h u) -> p h u", u=1).broadcast_to([128, H, H]),
                    )
                elif not _EDIRECT:
                    nc.vector.reciprocal(zr, zr)
                    a_bf = ap_.tile([128, H, H], bf16)
                    nc.vector.tensor_mul(
                        a_bf,
                        ex,
                        zr.rearrange("p (h u) -> p h u", u=1).broadcast_to([128, H, H]),
                    )
                else:
                    nc.vector.reciprocal(zr, zr)
                    a_bf = ex

                # ---- phase 2b: o = A @ v per token: product + tree over t
                o_c = op_.tile([128, E], bf16)  # [tok, (h d)]
                vt_b = vt.rearrange("p (u d) t -> p u d t", u=1)
                for g in range(_NH):
                    pav = pp.tile([128, HH, DH, H], bf16, tag="prod")
                    _a_src = mask_t if os.environ.get("KERNEL_FAKEA", "0") == "1" else a_bf
                    _a_ap = (_a_src.rearrange("p (a b t) -> p a b t", a=1, b=1)
                             .broadcast_to([128, HH, DH, H])) if _a_src is mask_t else (
                        a_bf[:, ds(g * HH, HH), :]
                        .rearrange("p h (u t) -> p h u t", u=1)
                        .broadcast_to([128, HH, DH, H]))
                    (nc.gpsimd if gp_tile else nc.vector).tensor_mul(
                        pav,
                        _a_ap,
                        vt_b.broadcast_to([128, HH, DH, H]),
                    )
                    if _SCAN:
                        nc.vector.tensor_tensor_scan(
                            pav.rearrange("p h d t -> p (h d) t"),
                            mask_t.rearrange("p (u t) -> p u t", u=1).broadcast_to(
                                [128, HH * DH, H]
                            ),
                            pav.rearrange("p h d t -> p (h d) t"),
                            0.0,
                            OP.mult,
                            OP.add,
                        )
                    else:
                        eng_a = nc.gpsimd if (gp_tile or (g == 1 and _GPMODE in (1, 3))) else nc.vector
                        w = H
                        while w > 1:
                            w //= 2
                            e_ = nc.gpsimd if (w < (1 << _GPT_AV)) else eng_a
                            e_.tensor_add(
                                pav[:, :, :, 0:w],
                                pav[:, :, :, 0:w],
                                pav[:, :, :, w : 2 * w],
                            )
                    if _EDIRECT:
                        for hh in range(HH):
                            nc.scalar.activation(
                                o_c[:, ds((g * HH + hh) * DH, DH)],
                                pav[:, hh, :, 0],
                                AF.Copy,
                                scale=zr[:, g * HH + hh : g * HH + hh + 1],
                            )
                    else:
                        _oc_eng = {"gpsimd": nc.gpsimd, "vector": nc.vector}.get(
                            os.environ.get("KERNEL_OCENG", "scalar"), nc.scalar
                        )
                        (_oc_eng.tensor_copy if _oc_eng is not nc.scalar else nc.scalar.copy)(
                            o_c[:, ds(g * HH * DH, HH * DH)].rearrange(
                                "p (h d) -> p h d", h=HH
                            ),
                            pav[:, :, :, H - 1] if _SCAN else pav[:, :, :, 0],
                        )

                # ---- transpose o to channel-major for o_proj
                pst = ps_t.tile([128, KC, 128], bf16)
                oT = op_.tile([128, KC, 128], bf16)
                ysb = yp.tile([128, E], f32)
                if os.environ.get("KERNEL_OPIPE", "0") == "1":
                    # interleave per contraction-half: transpose half -> copy ->
                    # its partial MMs, so the o_proj starts before the full o^T
                    # is materialized (shrinks the last tile's serial tail).
                    psy0 = ps_y.tile([128, 512], f32)
                    psy1 = ps_y.tile([128, 512], f32)
                    for hf in range(2):
                        for m in range(4 * hf, 4 * hf + 4):
                            nc.tensor.transpose(pst[:, m, :], o_c[:, ts(m, 128)], ident)
                        nc.scalar.copy(
                            oT[:, 4 * hf : 4 * hf + 4, :], pst[:, 4 * hf : 4 * hf + 4, :]
                        )
                        for j, psy in ((0, psy0), (1, psy1)):
                            for m in range(4 * hf, 4 * hf + 4):
                                nc.tensor.matmul(
                                    psy,
                                    oT[:, m, :],
                                    wo_sb[:, m, ds(j * 512, 512)],
                                    start=(m == 0),
                                    stop=False,
                                )
                    for j, psy in ((0, psy0), (1, psy1)):
                        nc.tensor.matmul(
                            psy,
                            ones[0:1, :],
                            bo1[0:1, ds(j * 512, 512)],
                            start=False,
                            stop=True,
                        )
                        nc.scalar.copy(ysb[:, ds(j * 512, 512)], psy)
                else:
                    for m in range(KC):
                        nc.tensor.transpose(pst[:, m, :], o_c[:, ts(m, 128)], ident)
                    nc.scalar.copy(oT[:, 0:4, :], pst[:, 0:4, :])
                    nc.scalar.copy(oT[:, 4:8, :], pst[:, 4:8, :])

                    # ---- phase 3: y = o @ Wo + bo
                    for j in range(2):
                        psy = ps_y.tile([128, 512], f32)
                        for m in range(KC):
                            nc.tensor.matmul(
                                psy,
                                oT[:, m, :],
                                wo_sb[:, m, ds(j * 512, 512)],
                                start=(m == 0),
                                stop=False,
                            )
                        nc.tensor.matmul(
                            psy,
                            ones[0:1, :],
                            bo1[0:1, ds(j * 512, 512)],
                            start=False,
                            stop=True,
                        )
                        nc.scalar.copy(ysb[:, ds(j * 512, 512)], psy)
                if os.environ.get("KERNEL_YSPLIT", "1") == "1":
                    nc.sync.dma_start(y_d[ts(i, PT), 0:512], ysb[:, 0:512])
                    nc.sync.dma_start(y_d[ts(i, PT), 512:1024], ysb[:, 512:1024])
                else:
                    nc.sync.dma_start(y_d[ts(i, PT), :], ysb)

            if _PIPE:
                qkv_next = phase1(0)
                for i in range(NT):
                    qkv_cur = qkv_next
                    if i + 1 < NT:
                        qkv_next = phase1(i + 1)
                    attention(i, qkv_cur)
            else:
                for i in range(NT):
                    attention(i, phase1(i))

    _legalize_waits(nc, mybir)
    return nc


def _legalize_waits(nc, mybir):
    """This walrus build allows only ONE sync wait per engine instruction.
    Split extra waits into standalone same-engine EventSemaphore insts."""
    for f in nc.m.functions:
        for b in f.blocks:
            newl = []
            for inst in b.instructions:
                si = getattr(inst, "sync_info", None)
                ow = list(si.on_wait) if si and si.on_wait else []
                if len(ow) > 1:
                    for w in ow[:-1]:
                        newl.append(
                            mybir.InstEventSemaphore(
                                name=f"WS-{nc.next_id()}",
                                engine=inst.engine,
                                sync_info=mybir.SyncInfo(on_wait=[w], on_update=[]),
                            )
                        )
                    si.on_wait = [ow[-1]]
                newl.append(inst)
            b.instructions = newl


def _prep_weights(w_qkv, b_qkv, w_o, b_o):
    # permute fused-qkv columns: orig e = h*192 + part*64 + d
    #                           new  e = part*1024 + h*64 + d
    part, h, d = np.meshgrid(
        np.arange(3), np.arange(H), np.arange(DH), indexing="ij"
    )
    perm = (h * 192 + part * 64 + d).reshape(-1)
    wq = np.ascontiguousarray(w_qkv[:, perm]).astype(ml_dtypes.bfloat16)
    bq = np.ascontiguousarray(b_qkv[perm]).astype(ml_dtypes.bfloat16)[None, :]
    wo = np.ascontiguousarray(w_o).astype(ml_dtypes.bfloat16)
    return (
        wq.reshape(KC, 128, 3 * E),
        bq,
        wo.reshape(KC, 128, E),
        np.asarray(b_o).astype(ml_dtypes.bfloat16)[None, :],
    )


def kernel(x, w_qkv, b_qkv, w_o, b_o):
    from concourse.bass_utils import run_bass_kernel_spmd

    if "nc" not in _CACHE:
        _CACHE["nc"] = _build_nc()
    nc = _CACHE["nc"]

    wq, bq, wo, bo = _prep_weights(
        np.asarray(w_qkv, np.float32),
        np.asarray(b_qkv, np.float32),
        np.asarray(w_o, np.float32),
        np.asarray(b_o, np.float32),
    )
    x = np.asarray(x, np.float32)
    in_maps = []
    for c in range(NCORES):
        xc = x[2 * c : 2 * c + 2].reshape(TOK, C)
        xT = np.ascontiguousarray(xc.T).astype(ml_dtypes.bfloat16)
        in_maps.append(
            {
                "xT": xT.reshape(KC, 128, TOK),
                "wqkv": wq,
                "bqkv": bq,
                "wo": wo,
                "bo": bo,
            }
        )

    res = run_bass_kernel_spmd(nc, in_maps, core_ids=list(range(NCORES)))
    out = np.empty((B, S, E), np.float32)
    for c in range(NCORES):
        out[2 * c : 2 * c + 2] = res.results[c]["y"].reshape(2, S, E)
    return out

